# revision 2
# baseline (speedup 1.0000x reference)
"""Causal self-attention (B=4, T=2048, C=1024, H=16) on 8 trn2 NeuronCores.

Sharding: core c handles batch b = c//2 and head-group hg = c%2 (8 of the 16
heads, i.e. 512 of the 1024 channels).  Each core computes its heads' QKV
projections, causal attention, and a *partial* out-projection over its 512
channels; the host sums the two partial outputs per batch (the "all-reduce" of
the row-sharded out_proj, done in numpy) and the hg==0 core adds bo.

Device-side layout is "transposed space": the host passes x[b].T so that the
contraction dim (channels) sits on SBUF partitions for every matmul:
  qT/kT [d, t] = W_hg @ x^T       (PE: lhsT = W^T chunk, rhs = x^T chunk)
  scoresT [s, q] = kT s-tile vs qT q-chunk (softmax dim on partitions)
  softmax-over-s via exp (ACT, scale=1/8) + ones-augmented V matmul:
  attn_out^T [65, q] = v_aug^T @ exp  (row 64 accumulates l = sum_s exp)
  normalize via PE outer-product broadcast of 1/l across partitions
  y[t, e] partial = aT^T @ Wo_hg^T
Causality at 128-token granularity by skipping upper s-tiles; the 4 diagonal
s-tiles of each 512-wide q-chunk are masked post-exp with precomputed 0/1
masks (only 4 distinct [128, 512] masks exist).

All big matmuls run in fp32r (full PE rate vs 4x-slow fp32).  walrus requires
fp32r matmul operands to be *produced* as fp32r: engine-produced tiles are
simply typed fp32r (the engine rounds), DMA-fed tiles are typed fp32r
end-to-end and the host pre-rounds the data (zero/round low 12 mantissa bits).
"""

import sys

if "/opt/trn_rl_repo" not in sys.path:
    sys.path.insert(0, "/opt/trn_rl_repo")

from contextlib import ExitStack

import numpy as np

import concourse.bass as bass
import concourse.tile as tile
from concourse import mybir
from concourse.bass_utils import run_bass_kernel_spmd

F32 = mybir.dt.float32
FR = mybir.dt.float32r

DMA_LOADS = "gpsimd"   # engine for bulk input loads: "sync" or "gpsimd"
DMA_STORES = "sync"    # engine for output stores
PS2_BUFS = 2           # scores psum tiles in flight (2 banks each)
PAV_BUFS = 2
BC_BUFS = 2
E_BUFS = 5
LOOKAHEAD = 4          # score-pairs emitted ahead of attn@v
XT0_FIRST = True       # first x chunk DMA'd before weights

B, T, C, H, HD = 4, 2048, 1024, 16, 64
HPC = 8            # heads per core
DC = HPC * HD      # channels per core = 512
NCORES = 8
NG = DC // 128     # 4 d-chunks of 128 (2 heads each)
NKC = C // 128     # 8 contraction chunks over C
NTC = T // 512     # 4 q/t-chunks of 512
NST = T // 128     # 16 s/t-tiles of 128

USE_FP32R = True   # fp32r = full-rate PE (1 cyc/row at N>=256) vs fp32 (4 cyc/row)
USE_BF16 = True    # bf16 operands: slightly faster PE (separate LDW), half DMA
BF16 = mybir.dt.bfloat16
MDT = BF16 if USE_BF16 else (FR if USE_FP32R else F32)


def round_fp32r(a):
    """Cast host data to the matmul operand dtype (fp32r rounding or bf16)."""
    if USE_BF16:
        import ml_dtypes
        return np.ascontiguousarray(
            np.ascontiguousarray(a, np.float32).astype(ml_dtypes.bfloat16))
    if not USE_FP32R:
        return np.ascontiguousarray(a, np.float32)
    bits = np.ascontiguousarray(a, np.float32).view(np.uint32)
    return (((bits.astype(np.uint64) + 0x800) & 0xFFFFF000)
            .astype(np.uint32).view(np.float32))


def build(reps=1):
    """Build the single-core Bass program (SPMD: all 8 cores run it).
    reps>1 repeats the whole body back-to-back in one NEFF (timing aid)."""
    nc = bass.Bass("TRN2", target_bir_lowering=False, debug=False)

    xT = nc.dram_tensor("xT", [NTC, NKC, 128, 512], MDT, kind="ExternalInput")
    wq = nc.dram_tensor("wq", [NKC, 128, DC], MDT, kind="ExternalInput")
    wk = nc.dram_tensor("wk", [NKC, 128, DC], MDT, kind="ExternalInput")
    wv = nc.dram_tensor("wv", [NKC, 128, DC], MDT, kind="ExternalInput")
    wo = nc.dram_tensor("wo", [NG, 128, C], MDT, kind="ExternalInput")
    bqk = nc.dram_tensor("bqk", [128, NG, 2], F32, kind="ExternalInput")
    bvb = nc.dram_tensor("bvb", [128, DC], F32, kind="ExternalInput")
    bob = nc.dram_tensor("bob", [128, C], F32, kind="ExternalInput")
    msk = nc.dram_tensor("msk", [128, 4, 512], F32, kind="ExternalInput")
    y = nc.dram_tensor("y", [T, C], F32, kind="ExternalOutput")

    EXP = mybir.ActivationFunctionType.Exp

    with tile.TileContext(nc) as tc:
      for _rep in range(reps):
        with tc.tile_pool(name="big", bufs=1) as big:
          _emit_body(nc, tc, big, locals())

    _split_matmul_waits(nc)
    return nc


def _emit_body(nc, tc, big, env):
    xT, wq, wk, wv, wo = env["xT"], env["wq"], env["wk"], env["wv"], env["wo"]
    bqk, bvb, bob, msk, y = env["bqk"], env["bvb"], env["bob"], env["msk"], env["y"]
    EXP = env["EXP"]
    if True:
        qT = big.tile([128, NG, T], MDT, tag="qT")   # [p, g, t], d = g*128+p
        kT = big.tile([128, NG, T], MDT, tag="kT")
        vv = big.tile([128, NST, HPC, HD + 1], MDT, tag="v")  # last col = 1.0

        # ---------------- phase 1: QKV projections ----------------
        with tc.tile_pool(name="ph1w", bufs=1) as ph1w, \
             tc.tile_pool(name="ph1x", bufs=2) as ph1x, \
             tc.tile_pool(name="ph1s", bufs=1) as ph1s, \
             tc.tile_pool(name="psum1", bufs=4, space="PSUM") as psum1:
            wq_sb = ph1w.tile([128, NKC, DC], MDT, tag="wq")
            wk_sb = ph1w.tile([128, NKC, DC], MDT, tag="wk")
            wv_sb = ph1w.tile([128, NKC, DC], MDT, tag="wv")
            bqk_sb = ph1s.tile([128, NG, 2], F32, tag="bqk")
            bvb_sb = ph1s.tile([128, DC], F32, tag="bvb")
            ld = getattr(nc, DMA_LOADS)
            # first x chunk goes FIRST (on SWDGE) while the weights stream in
            # parallel on the sync engine's HWDGE queues
            xt0 = None
            if XT0_FIRST:
                xt0 = ph1x.tile([128, NKC, 512], MDT, tag="xt")
                for k in range(NKC):
                    ld.dma_start(out=xt0[:, k], in_=xT[0, k])
            for k in range(NKC):
                nc.sync.dma_start(out=wq_sb[:, k], in_=wq[k])
                nc.sync.dma_start(out=wk_sb[:, k], in_=wk[k])
                nc.sync.dma_start(out=wv_sb[:, k], in_=wv[k])
            nc.sync.dma_start(out=bqk_sb, in_=bqk[:])
            nc.sync.dma_start(out=bvb_sb, in_=bvb[:])
            # memset doesn't accept fp32r APs; 1.0 is fp32r-exact, so poke
            # the same bytes through an fp32 view
            ones_col = vv[:, :, :, HD:HD + 1]
            nc.vector.memset(ones_col.bitcast(F32) if MDT == FR else ones_col, 1.0)

            for tci in range(NTC):
                tsl = slice(tci * 512, (tci + 1) * 512)
                if tci == 0 and xt0 is not None:
                    xt = xt0
                else:
                    xt = ph1x.tile([128, NKC, 512], MDT, tag="xt")
                    ld = getattr(nc, DMA_LOADS)
                    for k in range(NKC):
                        ld.dma_start(out=xt[:, k], in_=xT[tci, k])
                for si in range(4):
                    st = tci * 4 + si
                    ps = psum1.tile([128, 512], F32, tag="ps1")
                    for k in range(NKC):
                        nc.tensor.matmul(
                            ps,
                            xt[:, k, si * 128:(si + 1) * 128],
                            wv_sb[:, k],
                            start=(k == 0), stop=(k == NKC - 1))
                    nc.vector.tensor_add(
                        vv[:, st, :, 0:HD],
                        ps.rearrange("p (h d) -> p h d", h=HPC),
                        bvb_sb.rearrange("p (h d) -> p h d", h=HPC))
                for (w_sb, bcol, dst) in ((wq_sb, 0, qT), (wk_sb, 1, kT)):
                    for g in range(NG):
                        ps = psum1.tile([128, 512], F32, tag="ps1")
                        for k in range(NKC):
                            nc.tensor.matmul(
                                ps,
                                w_sb[:, k, g * 128:(g + 1) * 128],
                                xt[:, k],
                                start=(k == 0), stop=(k == NKC - 1))
                        nc.scalar.activation(
                            out=dst[:, g, tsl], in_=ps,
                            func=mybir.ActivationFunctionType.Identity,
                            bias=bqk_sb[:, g, bcol:bcol + 1])

        # ---------------- phases 2+3 ----------------
        with tc.tile_pool(name="late", bufs=1) as late:
            wo_sb = late.tile([128, NG, C], MDT, tag="wo")
            bob_sb = late.tile([128, C], F32, tag="bob")
            aT = late.tile([128, NG, T], MDT, tag="aT")
            for g in range(NG):
                nc.sync.dma_start(out=wo_sb[:, g], in_=wo[g])
            nc.sync.dma_start(out=bob_sb, in_=bob[:])

            # -------- phase 2: attention per head --------
            with tc.tile_pool(name="ph2s", bufs=1) as ph2s, \
                 tc.tile_pool(name="ph2e", bufs=E_BUFS) as ph2e, \
                 tc.tile_pool(name="ph2t", bufs=2) as ph2t, \
                 tc.tile_pool(name="psum_s", bufs=PS2_BUFS, space="PSUM") as psum_s, \
                 tc.tile_pool(name="psum_av", bufs=PAV_BUFS, space="PSUM") as psum_av, \
                 tc.tile_pool(name="psum_bc", bufs=BC_BUFS, space="PSUM") as psum_bc:
                msk_sb = ph2s.tile([128, 4, 512], F32, tag="msk")
                ones_sb = ph2s.tile([128, HD], MDT, tag="ones")
                getattr(nc, DMA_LOADS).dma_start(out=msk_sb, in_=msk[:])
                nc.vector.memset(ones_sb.bitcast(F32) if MDT == FR else ones_sb, 1.0)

                def emit_norm(pav, g, po, qs):
                    # normalize: aT[:, q] = pav[0:64, q] / pav[64, q]
                    lr = ph2t.tile([HD + 1, 512], MDT, tag="lr")
                    with nc.allow_low_precision(reason="1/l bcast via fp32r matmul"):
                        nc.vector.reciprocal(lr[HD:HD + 1], pav[HD:HD + 1])
                    bc = psum_bc.tile([HD, 512], F32, tag="bc")
                    nc.tensor.matmul(bc, ones_sb[HD:HD + 1, :], lr[HD:HD + 1],
                                     start=True, stop=True, skip_group_check=True)
                    # DVE can read only ONE PSUM operand; stage bc in SBUF
                    bc_sb = ph2t.tile([HD, 512], F32, tag="bc_sb")
                    nc.vector.tensor_copy(bc_sb, bc)
                    if po == 0:
                        nc.vector.tensor_mul(aT[0:HD, g, qs], pav[0:HD], bc_sb)
                    else:
                        tmp = ph2t.tile([HD, 512], MDT, tag="tmp")
                        nc.vector.tensor_mul(tmp, pav[0:HD], bc_sb)
                        # DVE cannot shift partitions; DMA moves 0:64 -> 64:128
                        getattr(nc, DMA_STORES).dma_start(out=aT[HD:128, g, qs], in_=tmp)

                pending_norm = []
                for hp in range(HPC // 2):
                    g = hp                      # head pair (2g, 2g+1)
                    for qc in range(NTC):
                        qs = slice(qc * 512, (qc + 1) * 512)
                        n_st = 4 * (qc + 1)
                        pavs = [psum_av.tile([HD + 1, 512], F32, tag="pav",
                                              name=f"pav{hp}_{qc}_{i}")
                                for i in range(2)]

                        def emit_av(e_pair, pair, hi):
                            for j in range(2):
                                st = pair * 2 + j
                                nc.tensor.matmul(
                                    pavs[hi],
                                    vv[:, st, 2 * g + hi],
                                    e_pair[:, j],
                                    start=(st == 0), stop=(st == n_st - 1),
                                    skip_group_check=True)

                        pending = []
                        for pair in range(n_st // 2):
                            # the two heads sit on PE row-groups 0-63 / 64-127:
                            # alternating them lets each self-loading LDW
                            # overlap the other head's in-flight matmul
                            es = []
                            for hi in range(2):
                                po = hi * HD
                                ps = psum_s.tile([128, 2, 512], F32, tag="ps2")
                                for j in range(2):
                                    st = pair * 2 + j
                                    nc.tensor.matmul(
                                        ps[:, j],
                                        kT[po:po + HD, g, st * 128:(st + 1) * 128],
                                        qT[po:po + HD, g, qs],
                                        start=True, stop=True,
                                        skip_group_check=True)
                                e = ph2e.tile([128, 2, 512], MDT, tag="e")
                                nc.scalar.activation(out=e, in_=ps, func=EXP,
                                                     scale=0.125)
                                for j in range(2):
                                    kk = pair * 2 + j - 4 * qc
                                    if kk >= 0:
                                        nc.vector.tensor_mul(e[:, j], e[:, j],
                                                             msk_sb[:, kk])
                                es.append(e)
                            pending.append((es, pair))
                            if pair == 0:
                                while pending_norm:
                                    emit_norm(*pending_norm.pop(0))
                            if len(pending) > LOOKAHEAD:
                                es_, pair_ = pending.pop(0)
                                emit_av(es_[0], pair_, 0)
                                emit_av(es_[1], pair_, 1)
                        for es_, pair_ in pending:
                            emit_av(es_[0], pair_, 0)
                            emit_av(es_[1], pair_, 1)
                        pending_norm.append((pavs[0], g, 0, qs))
                        pending_norm.append((pavs[1], g, HD, qs))
                for pn in pending_norm:
                    emit_norm(*pn)

            # -------- phase 3: partial out-projection --------
            with tc.tile_pool(name="ph3", bufs=3) as ph3, \
                 tc.tile_pool(name="psum3", bufs=3, space="PSUM") as psum3:
                for tt in range(NST):
                    tsl = slice(tt * 128, (tt + 1) * 128)
                    po_ = psum3.tile([128, C], F32, tag="po")
                    for g in range(NG):
                        for eh in range(2):
                            nc.tensor.matmul(
                                po_[:, eh * 512:(eh + 1) * 512],
                                aT[:, g, tsl],
                                wo_sb[:, g, eh * 512:(eh + 1) * 512],
                                start=(g == 0), stop=(g == NG - 1),
                                skip_group_check=True)
                    ot = ph3.tile([128, C], F32, tag="ot")
                    nc.vector.tensor_add(ot, po_, bob_sb)
                    getattr(nc, DMA_STORES).dma_start(out=y[tsl], in_=ot)


def _split_matmul_waits(nc):
    """walrus codegen allows only ONE sync-wait per engine instruction.
    Move surplus waits of any multi-wait instruction onto preceding
    same-engine NoOps (one wait each) — engine dispatch is in-order, so
    the NoOps gate the instruction."""
    from concourse import mybir

    inst_noop_cls = None
    for fn in nc.m.functions:
        for blk in fn.blocks:
            new_insts = []
            for inst in blk.instructions:
                si = getattr(inst, "sync_info", None)
                if (si is not None
                        and si.on_wait and len(si.on_wait) > 1):
                    if inst_noop_cls is None:
                        import bass_rust
                        inst_noop_cls = bass_rust.InstNoOp
                    waits = list(si.on_wait)
                    si.on_wait = waits[-1:]
                    for w in waits[:-1]:  # one wait per NoOp (HW limit)
                        nop = inst_noop_cls(
                            name=nc.get_next_instruction_name(), ins=[], outs=[])
                        nop.engine = inst.engine
                        nop.sync_info = mybir.SyncInfo(on_wait=[w], on_update=[])
                        nc.register_instruction(nop)
                        new_insts.append(nop)
                new_insts.append(inst)
            blk.instructions[:] = new_insts


def prepare_inputs(inputs):
    """Per-core input maps (host-side sharding + layout munging)."""
    x = np.asarray(inputs["x"], np.float32)
    Wq = np.asarray(inputs["Wq"], np.float32)
    bq = np.asarray(inputs["bq"], np.float32)
    Wk = np.asarray(inputs["Wk"], np.float32)
    bk = np.asarray(inputs["bk"], np.float32)
    Wv = np.asarray(inputs["Wv"], np.float32)
    bv = np.asarray(inputs["bv"], np.float32)
    Wo = np.asarray(inputs["Wo"], np.float32)
    bo = np.asarray(inputs["bo"], np.float32)

    p = np.arange(128)[:, None, None]
    kk = np.arange(4)[None, :, None]
    f = np.arange(512)[None, None, :]
    msk = ((p + 128 * kk) <= f).astype(np.float32)  # [128, 4, 512]

    in_maps = []
    for c in range(NCORES):
        b, hg = c // 2, c % 2
        rows = slice(hg * DC, (hg + 1) * DC)
        in_maps.append({
            "xT": np.ascontiguousarray(round_fp32r(x[b].T).reshape(NKC, 128, NTC, 512).transpose(2, 0, 1, 3)),
            "wq": round_fp32r(Wq[rows, :].T).reshape(NKC, 128, DC),
            "wk": round_fp32r(Wk[rows, :].T).reshape(NKC, 128, DC),
            "wv": round_fp32r(Wv[rows, :].T).reshape(NKC, 128, DC),
            "wo": round_fp32r(Wo[:, rows].T).reshape(NG, 128, C),
            "bqk": np.ascontiguousarray(
                np.stack([bq[rows].reshape(NG, 128).T,
                          bk[rows].reshape(NG, 128).T], axis=-1)),
            "bvb": np.tile(bv[rows][None, :], (128, 1)),
            "bob": (np.tile(bo[None, :], (128, 1)) if hg == 0
                    else np.zeros((128, C), np.float32)),
            "msk": msk,
        })
    return in_maps


def gather_outputs(results):
    ys = [np.asarray(r["y"], np.float32) for r in results]
    return np.stack([ys[2 * b] + ys[2 * b + 1] for b in range(B)], axis=0)


def kernel(**inputs):
    nc = build()
    in_maps = prepare_inputs(inputs)
    res = run_bass_kernel_spmd(nc, in_maps, core_ids=list(range(NCORES)))
    return gather_outputs(res.results)



# revision 7
# speedup vs baseline: 1.1589x; 1.1589x over previous
"""Causal self-attention (B=4, T=2048, C=1024, H=16) on 8 trn2 NeuronCores.

Sharding: core c handles batch b = c//2 and head-group hg = c%2 (8 of the 16
heads, i.e. 512 of the 1024 channels).  Each core computes its heads' QKV
projections, causal attention, and a *partial* out-projection over its 512
channels; the host sums the two partial outputs per batch (the "all-reduce" of
the row-sharded out_proj, done in numpy) and the hg==0 core adds bo.

v2 design (vs the phased v1):
 - bf16 matmul operands everywhere (fp32 PSUM accumulate); host pre-casts.
 - single fused pipeline per 512-token t-chunk: QKV projections, then the
   previous chunk's out-projection, then attention for all 4 head pairs,
   with a FIFO backlog deferring attn@V matmuls a few slots behind the
   score matmuls so the scalar-engine exp latency is hidden.
 - scores run as K=128 matmuls with zero-padded per-head keys
   (kT[:, g, 0] = [k_headA; 0], kT[:, g, 1] = [0; k_headB]) so every matmul
   in the kernel uses the full 128x128 PE configuration - no tiling-mode
   switches, and q needs no padding since the zero rows annihilate it.
 - causal diagonal tiles shrink N: score/exp/attn@V only cover the
   unmasked q-suffix; the masked prefix of e is memset to 0 and the single
   triangular 128x128 block is masked with one shared bf16 mask tile.
 - k-bias is dropped: scores' + (bk.q)[q] is constant across s for fixed q,
   and softmax is invariant to per-q shifts (exact identity).
 - softmax normalization: l comes from the ones-column of the augmented V
   (row 64 of pav); 1/l is broadcast across partitions by the otherwise-idle
   GpSimd engine (partition_broadcast) instead of a PE outer-product.
"""

import sys

if "/opt/trn_rl_repo" not in sys.path:
    sys.path.insert(0, "/opt/trn_rl_repo")

import numpy as np

import concourse.bass as bass
import concourse.tile as tile
from concourse import mybir
from concourse.bass_utils import run_bass_kernel_spmd

F32 = mybir.dt.float32
BF16 = mybir.dt.bfloat16
MDT = BF16

B, T, C, H, HD = 4, 2048, 1024, 16, 64
HPC = 8            # heads per core
DC = HPC * HD      # channels per core = 512
NCORES = 8
NG = DC // 128     # 4 d-chunks of 128 (2 heads each)
NKC = C // 128     # 8 contraction chunks over C
NTC = T // 512     # 4 q/t-chunks of 512
NST = T // 128     # 16 s/t-tiles of 128

LOOKAHEAD = 3      # attn@V slots deferred behind score slots
E_BUFS = 6

EXPF = mybir.ActivationFunctionType.Exp
IDN = mybir.ActivationFunctionType.Identity


def build(reps=1):
    nc = bass.Bass("TRN2", target_bir_lowering=False, debug=False)

    xT = nc.dram_tensor("xT", [NTC, NKC, 128, 512], MDT, kind="ExternalInput")
    wq = nc.dram_tensor("wq", [NKC, 128, DC], MDT, kind="ExternalInput")
    wk = nc.dram_tensor("wk", [NKC, 128, DC], MDT, kind="ExternalInput")
    wv = nc.dram_tensor("wv", [NKC, 128, DC], MDT, kind="ExternalInput")
    wo = nc.dram_tensor("wo", [NG, 128, C], MDT, kind="ExternalInput")
    bqt = nc.dram_tensor("bqt", [128, NG], F32, kind="ExternalInput")
    bvb = nc.dram_tensor("bvb", [128, DC], F32, kind="ExternalInput")
    bob = nc.dram_tensor("bob", [128, C], F32, kind="ExternalInput")
    msk = nc.dram_tensor("msk", [128, 128], MDT, kind="ExternalInput")
    y = nc.dram_tensor("y", [T, C], F32, kind="ExternalOutput")

    env = dict(xT=xT, wq=wq, wk=wk, wv=wv, wo=wo, bqt=bqt, bvb=bvb,
               bob=bob, msk=msk, y=y)

    with tile.TileContext(nc) as tc:
        for _rep in range(reps):
            with tc.tile_pool(name="big", bufs=1) as big:
                _emit_body(nc, tc, big, env)

    _split_matmul_waits(nc)
    return nc


def _emit_body(nc, tc, big, env):
    xT, wq, wk, wv, wo = env["xT"], env["wq"], env["wk"], env["wv"], env["wo"]
    bqt, bvb, bob, msk, y = env["bqt"], env["bvb"], env["bob"], env["msk"], env["y"]

    qT = big.tile([128, NG, T], MDT, tag="qT")         # [p, g, t], d = g*128+p
    kT = big.tile([128, NG, 2, T], MDT, tag="kT")      # per-head zero-padded
    vv = big.tile([128, NST, HPC, HD + 1], MDT, tag="v")  # last col = 1.0
    aT = big.tile([128, NG, T], MDT, tag="aT")
    wq_sb = big.tile([128, NKC, DC], MDT, tag="wq")
    wk_sb = big.tile([128, NKC, DC], MDT, tag="wk")
    wv_sb = big.tile([128, NKC, DC], MDT, tag="wv")
    wo_sb = big.tile([128, NG, C], MDT, tag="wo")
    bqt_sb = big.tile([128, NG], F32, tag="bqt")
    bvb_sb = big.tile([128, DC], F32, tag="bvb")
    bob_sb = big.tile([128, C], F32, tag="bob")
    msk_sb = big.tile([128, 128], MDT, tag="msk")

    with tc.tile_pool(name="ph1x", bufs=2) as ph1x, \
         tc.tile_pool(name="pe_", bufs=E_BUFS) as pe_, \
         tc.tile_pool(name="pt_", bufs=2) as pt_, \
         tc.tile_pool(name="pot", bufs=2) as pot, \
         tc.tile_pool(name="psc", bufs=2, space="PSUM") as psc, \
         tc.tile_pool(name="ppav", bufs=2, space="PSUM") as ppav, \
         tc.tile_pool(name="pmm", bufs=2, space="PSUM") as pmm:

        # ------------- initial DMAs + one-time init -------------
        xt0 = ph1x.tile([128, NKC, 512], MDT, tag="xt", name="xt0")
        for k in range(NKC):
            nc.gpsimd.dma_start(out=xt0[:, k], in_=xT[0, k])
        for k in range(NKC):
            nc.sync.dma_start(out=wv_sb[:, k], in_=wv[k])
        for k in range(NKC):
            nc.sync.dma_start(out=wq_sb[:, k], in_=wq[k])
            nc.sync.dma_start(out=wk_sb[:, k], in_=wk[k])
        for g in range(NG):
            nc.sync.dma_start(out=wo_sb[:, g], in_=wo[g])
        nc.sync.dma_start(out=bqt_sb, in_=bqt[:])
        nc.sync.dma_start(out=bvb_sb, in_=bvb[:])
        nc.sync.dma_start(out=bob_sb, in_=bob[:])
        nc.sync.dma_start(out=msk_sb, in_=msk[:])
        nc.vector.memset(vv[:, :, :, HD:HD + 1], 1.0)
        nc.vector.memset(kT[64:128, :, 0, :], 0.0)
        nc.vector.memset(kT[0:64, :, 1, :], 0.0)
        ones_sb = big.tile([128, HD], MDT, tag="ones")
        nc.vector.memset(ones_sb, 1.0)

        backlog = []

        def drain(n=1):
            for _ in range(min(n, len(backlog))):
                backlog.pop(0)()

        def emit_ph1(tci, xt):
            tsl = slice(tci * 512, (tci + 1) * 512)
            for si in range(4):
                st = tci * 4 + si
                psv = pmm.tile([128, 512], F32, tag="mm", name=f"psv{st}")
                for k in range(NKC):
                    nc.tensor.matmul(
                        psv, xt[:, k, si * 128:(si + 1) * 128], wv_sb[:, k],
                        start=(k == 0), stop=(k == NKC - 1),
                        skip_group_check=True)
                nc.vector.tensor_add(
                    vv[:, st, :, 0:HD],
                    psv.rearrange("p (h d) -> p h d", h=HPC),
                    bvb_sb.rearrange("p (h d) -> p h d", h=HPC))
                drain(1)
            for g in range(NG):
                psq = pmm.tile([128, 512], F32, tag="mm", name=f"psq{tci}_{g}")
                for k in range(NKC):
                    nc.tensor.matmul(
                        psq, wq_sb[:, k, g * 128:(g + 1) * 128], xt[:, k],
                        start=(k == 0), stop=(k == NKC - 1),
                        skip_group_check=True)
                nc.scalar.activation(out=qT[:, g, tsl], in_=psq, func=IDN,
                                     bias=bqt_sb[:, g:g + 1])
                drain(1)
                psk = pmm.tile([128, 512], F32, tag="mm", name=f"psk{tci}_{g}")
                for k in range(NKC):
                    nc.tensor.matmul(
                        psk, wk_sb[:, k, g * 128:(g + 1) * 128], xt[:, k],
                        start=(k == 0), stop=(k == NKC - 1),
                        skip_group_check=True)
                # k-bias dropped: softmax is invariant to per-q score shifts
                nc.vector.tensor_copy(kT[0:64, g, 0, tsl], psk[0:64])
                nc.vector.tensor_copy(kT[64:128, g, 1, tsl], psk[64:128])
                drain(1)

        def emit_ph3(tt):
            tsl = slice(tt * 128, (tt + 1) * 128)
            po0 = pmm.tile([128, 512], F32, tag="mm", name=f"po0_{tt}")
            po1 = pmm.tile([128, 512], F32, tag="mm", name=f"po1_{tt}")
            for g in range(NG):
                nc.tensor.matmul(po0, aT[:, g, tsl], wo_sb[:, g, 0:512],
                                 start=(g == 0), stop=(g == NG - 1),
                                 skip_group_check=True)
            drain(1)
            for g in range(NG):
                nc.tensor.matmul(po1, aT[:, g, tsl], wo_sb[:, g, 512:1024],
                                 start=(g == 0), stop=(g == NG - 1),
                                 skip_group_check=True)
            ot = pot.tile([128, C], F32, tag="ot", name=f"ot{tt}")
            nc.vector.tensor_add(ot[:, 0:512], po0, bob_sb[:, 0:512])
            nc.vector.tensor_add(ot[:, 512:1024], po1, bob_sb[:, 512:1024])
            nc.sync.dma_start(out=y[tsl], in_=ot)
            drain(1)

        def emit_attn(g, qc):
            qcs = qc * 512
            n_st = 4 * (qc + 1)
            pavs = [ppav.tile([HD + 1, 512], F32, tag="pav",
                              name=f"pav{g}_{qc}_{hi}") for hi in range(2)]

            def emit_pav(e, st, off):
                for hi in range(2):
                    nc.tensor.matmul(
                        pavs[hi][:, off:], vv[:, st, 2 * g + hi],
                        e[:, hi, off:],
                        start=(st == 0), stop=(st == n_st - 1),
                        skip_group_check=True)

            def emit_norm(pav, hi):
                lr = pt_.tile([128, 512], MDT, tag="lr",
                              name=f"lr{g}_{qc}_{hi}")
                with nc.allow_low_precision(reason="1/l in bf16"):
                    nc.vector.reciprocal(lr[HD:HD + 1], pav[HD:HD + 1])
                # broadcast 1/l across partitions via PE outer product
                bc = pmm.tile([128, 512], F32, tag="mm",
                              name=f"bc{g}_{qc}_{hi}")
                nc.tensor.matmul(bc[0:HD], ones_sb[HD:HD + 1, :],
                                 lr[HD:HD + 1], start=True, stop=True,
                                 skip_group_check=True)
                bc_sb = pt_.tile([HD, 512], F32, tag="bc_sb",
                                 name=f"bcs{g}_{qc}_{hi}")
                nc.scalar.copy(bc_sb, bc[0:HD])
                if hi == 0:
                    nc.vector.tensor_mul(aT[0:HD, g, qcs:qcs + 512],
                                         pav[0:HD], bc_sb)
                else:
                    tmp = pt_.tile([HD, 512], MDT, tag="tmp",
                                   name=f"tmp{g}_{qc}")
                    nc.vector.tensor_mul(tmp, pav[0:HD], bc_sb)
                    nc.sync.dma_start(out=aT[HD:128, g, qcs:qcs + 512],
                                      in_=tmp)

            for st in range(n_st):
                kk = st - 4 * qc
                off = 128 * kk if kk >= 0 else 0
                sc = psc.tile([128, 2, 512], F32, tag="sc",
                              name=f"sc{g}_{qc}_{st}")
                for hi in range(2):
                    nc.tensor.matmul(
                        sc[:, hi, off:], kT[:, g, hi, st * 128:(st + 1) * 128],
                        qT[:, g, qcs + off:qcs + 512],
                        start=True, stop=True, skip_group_check=True)
                e = pe_.tile([128, 2, 512], MDT, tag="e",
                             name=f"e{g}_{qc}_{st}")
                nc.scalar.activation(out=e[:, :, off:], in_=sc[:, :, off:],
                                     func=EXPF, scale=0.125)
                if kk >= 0:
                    if off:
                        nc.vector.memset(e[:, :, 0:off], 0.0)
                    for hi in range(2):
                        nc.vector.tensor_mul(e[:, hi, off:off + 128],
                                             e[:, hi, off:off + 128], msk_sb)
                backlog.append(
                    lambda e=e, st=st, off=off: emit_pav(e, st, off))
                drain(1)
            backlog.append(lambda pav=pavs[0]: emit_norm(pav, 0))
            backlog.append(lambda pav=pavs[1]: emit_norm(pav, 1))

        # ---------------- fused pipeline ----------------
        for tci in range(NTC):
            if tci == 0:
                xt = xt0
            else:
                xt = ph1x.tile([128, NKC, 512], MDT, tag="xt",
                               name=f"xt{tci}")
                for k in range(NKC):
                    nc.gpsimd.dma_start(out=xt[:, k], in_=xT[tci, k])
            emit_ph1(tci, xt)
            if tci > 0:
                for tt in range(4 * (tci - 1), 4 * tci):
                    emit_ph3(tt)
            for g in range(NG):
                emit_attn(g, tci)
        while backlog:
            drain(1)
        for tt in range(4 * (NTC - 1), 4 * NTC):
            emit_ph3(tt)


def _split_matmul_waits(nc):
    """walrus codegen allows only ONE sync-wait per engine instruction.
    Move surplus waits of any multi-wait instruction onto preceding
    same-engine NoOps (one wait each) — engine dispatch is in-order, so
    the NoOps gate the instruction."""
    from concourse import mybir

    inst_noop_cls = None
    for fn in nc.m.functions:
        for blk in fn.blocks:
            new_insts = []
            for inst in blk.instructions:
                si = getattr(inst, "sync_info", None)
                if (si is not None
                        and si.on_wait and len(si.on_wait) > 1):
                    if inst_noop_cls is None:
                        import bass_rust
                        inst_noop_cls = bass_rust.InstNoOp
                    waits = list(si.on_wait)
                    si.on_wait = waits[-1:]
                    for w in waits[:-1]:  # one wait per NoOp (HW limit)
                        nop = inst_noop_cls(
                            name=nc.get_next_instruction_name(), ins=[], outs=[])
                        nop.engine = inst.engine
                        nop.sync_info = mybir.SyncInfo(on_wait=[w], on_update=[])
                        nc.register_instruction(nop)
                        new_insts.append(nop)
                new_insts.append(inst)
            blk.instructions[:] = new_insts


def to_bf16(a):
    import ml_dtypes
    return np.ascontiguousarray(
        np.ascontiguousarray(a, np.float32).astype(ml_dtypes.bfloat16))


def prepare_inputs(inputs):
    """Per-core input maps (host-side sharding + layout munging)."""
    x = np.asarray(inputs["x"], np.float32)
    Wq = np.asarray(inputs["Wq"], np.float32)
    bq = np.asarray(inputs["bq"], np.float32)
    Wk = np.asarray(inputs["Wk"], np.float32)
    Wv = np.asarray(inputs["Wv"], np.float32)
    bv = np.asarray(inputs["bv"], np.float32)
    Wo = np.asarray(inputs["Wo"], np.float32)
    bo = np.asarray(inputs["bo"], np.float32)

    p = np.arange(128)[:, None]
    f = np.arange(128)[None, :]
    msk = to_bf16((p <= f).astype(np.float32))  # [128, 128] triangular

    in_maps = []
    for c in range(NCORES):
        b, hg = c // 2, c % 2
        rows = slice(hg * DC, (hg + 1) * DC)
        in_maps.append({
            "xT": np.ascontiguousarray(
                to_bf16(x[b].T).reshape(NKC, 128, NTC, 512)
                .transpose(2, 0, 1, 3)),
            "wq": to_bf16(Wq[rows, :].T).reshape(NKC, 128, DC),
            "wk": to_bf16(Wk[rows, :].T).reshape(NKC, 128, DC),
            "wv": to_bf16(Wv[rows, :].T).reshape(NKC, 128, DC),
            "wo": to_bf16(Wo[:, rows].T).reshape(NG, 128, C),
            "bqt": np.ascontiguousarray(bq[rows].reshape(NG, 128).T),
            "bvb": np.tile(bv[rows][None, :], (128, 1)),
            "bob": (np.tile(bo[None, :], (128, 1)) if hg == 0
                    else np.zeros((128, C), np.float32)),
            "msk": msk,
        })
    return in_maps


def gather_outputs(results):
    ys = [np.asarray(r["y"], np.float32) for r in results]
    return np.stack([ys[2 * b] + ys[2 * b + 1] for b in range(B)], axis=0)


def kernel(**inputs):
    nc = build()
    in_maps = prepare_inputs(inputs)
    res = run_bass_kernel_spmd(nc, in_maps, core_ids=list(range(NCORES)))
    return gather_outputs(res.results)


# revision 14
# speedup vs baseline: 1.3379x; 1.1545x over previous
"""Causal self-attention (B=4, T=2048, C=1024, H=16) on 8 trn2 NeuronCores.

Sharding: core c handles batch b = c//2 and head-group hg = c%2 (8 of the 16
heads, i.e. 512 of the 1024 channels).  Each core computes its heads' QKV
projections, causal attention, and a *partial* out-projection over its 512
channels; the host sums the two partial outputs per batch (the "all-reduce" of
the row-sharded out_proj, done in numpy) and the hg==0 core adds bo.

v2 design (vs the phased v1):
 - bf16 matmul operands everywhere (fp32 PSUM accumulate); host pre-casts.
 - single fused pipeline per 512-token t-chunk: QKV projections, then the
   previous chunk's out-projection, then attention for all 4 head pairs,
   with a FIFO backlog deferring attn@V matmuls a few slots behind the
   score matmuls so the scalar-engine exp latency is hidden.
 - scores run as K=128 matmuls with zero-padded per-head keys
   (kT[:, g, 0] = [k_headA; 0], kT[:, g, 1] = [0; k_headB]) so every matmul
   in the kernel uses the full 128x128 PE configuration - no tiling-mode
   switches, and q needs no padding since the zero rows annihilate it.
 - causal diagonal tiles shrink N: score/exp/attn@V only cover the
   unmasked q-suffix; the masked prefix of e is memset to 0 and the single
   triangular 128x128 block is masked with one shared bf16 mask tile.
 - k-bias is dropped: scores' + (bk.q)[q] is constant across s for fixed q,
   and softmax is invariant to per-q shifts (exact identity).
 - softmax normalization: l comes from the ones-column of the augmented V
   (row 64 of pav); 1/l is broadcast across partitions by the otherwise-idle
   GpSimd engine (partition_broadcast) instead of a PE outer-product.
"""

import sys

if "/opt/trn_rl_repo" not in sys.path:
    sys.path.insert(0, "/opt/trn_rl_repo")

import numpy as np

import concourse.bass as bass
import concourse.tile as tile
from concourse import mybir
from concourse.bass_utils import run_bass_kernel_spmd

F32 = mybir.dt.float32
BF16 = mybir.dt.bfloat16
MDT = BF16

B, T, C, H, HD = 4, 2048, 1024, 16, 64
HPC = 8            # heads per core
DC = HPC * HD      # channels per core = 512
NCORES = 8
NG = DC // 128     # 4 d-chunks of 128 (2 heads each)
NKC = C // 128     # 8 contraction chunks over C
NTC = T // 512     # 4 q/t-chunks of 512
NST = T // 128     # 16 s/t-tiles of 128

import os as _os

LOOKAHEAD = int(_os.environ.get("KLOOKAHEAD", "3"))
E_BUFS = int(_os.environ.get("KEBUFS", "6"))
SCORES_RT = _os.environ.get("KSCORES_RT", "0") == "1"

EXPF = mybir.ActivationFunctionType.Exp
IDN = mybir.ActivationFunctionType.Identity


def build(reps=1):
    nc = bass.Bass("TRN2", target_bir_lowering=False, debug=False)

    xT = nc.dram_tensor("xT", [NTC, NKC, 128, 512], MDT, kind="ExternalInput")
    wq = nc.dram_tensor("wq", [NKC, 128, DC], MDT, kind="ExternalInput")
    wk = nc.dram_tensor("wk", [NKC, 128, DC], MDT, kind="ExternalInput")
    wv = nc.dram_tensor("wv", [NKC, 128, DC], MDT, kind="ExternalInput")
    wo = nc.dram_tensor("wo", [NG, 128, C], MDT, kind="ExternalInput")
    bqt = nc.dram_tensor("bqt", [128, NG], F32, kind="ExternalInput")
    bvb = nc.dram_tensor("bvb", [128, DC], F32, kind="ExternalInput")
    bob = nc.dram_tensor("bob", [128, C], F32, kind="ExternalInput")
    msk = nc.dram_tensor("msk", [128, 128], MDT, kind="ExternalInput")
    y = nc.dram_tensor("y", [T, C], F32, kind="ExternalOutput")

    env = dict(xT=xT, wq=wq, wk=wk, wv=wv, wo=wo, bqt=bqt, bvb=bvb,
               bob=bob, msk=msk, y=y)

    with tile.TileContext(nc) as tc:
        for _rep in range(reps):
            with tc.tile_pool(name="big", bufs=1) as big:
                _emit_body(nc, tc, big, env)

    _split_matmul_waits(nc)
    return nc


def _emit_body(nc, tc, big, env):
    xT, wq, wk, wv, wo = env["xT"], env["wq"], env["wk"], env["wv"], env["wo"]
    bqt, bvb, bob, msk, y = env["bqt"], env["bvb"], env["bob"], env["msk"], env["y"]

    qT = big.tile([128, NG, T], MDT, tag="qT")         # [p, g, t], d = g*128+p
    kT = big.tile([128, NG, 2, T], MDT, tag="kT")      # per-head zero-padded
    vv = big.tile([128, NST, HPC, HD + 1], MDT, tag="v")  # last col = 1.0
    aT = big.tile([128, NG, T], MDT, tag="aT")
    wq_sb = big.tile([128, NKC, DC], MDT, tag="wq")
    wk_sb = big.tile([128, NKC, DC], MDT, tag="wk")
    wv_sb = big.tile([128, NKC, DC], MDT, tag="wv")
    wo_sb = big.tile([128, NG, C], MDT, tag="wo")
    bqt_sb = big.tile([128, NG], F32, tag="bqt")
    bvb_sb = big.tile([128, DC], F32, tag="bvb")
    bob_sb = big.tile([128, C], F32, tag="bob")
    msk_sb = big.tile([128, 128], MDT, tag="msk")

    with tc.tile_pool(name="ph1x", bufs=2) as ph1x, \
         tc.tile_pool(name="pe_", bufs=E_BUFS) as pe_, \
         tc.tile_pool(name="pt_", bufs=2) as pt_, \
         tc.tile_pool(name="pot", bufs=2) as pot, \
         tc.tile_pool(name="psc", bufs=2, space="PSUM") as psc, \
         tc.tile_pool(name="ppav", bufs=2, space="PSUM") as ppav, \
         tc.tile_pool(name="pmm", bufs=2, space="PSUM") as pmm:

        # ------------- initial DMAs + one-time init -------------
        # xt0 + wv interleaved per-k across the two DMA paths so the first
        # v-chain can start after ~1 transfer each; wq/wk follow split
        # across both queues so the q/k chains aren't starved either.
        xt0 = ph1x.tile([128, NKC, 512], MDT, tag="xt", name="xt0")
        for k in range(NKC):
            nc.gpsimd.dma_start(out=xt0[:, k], in_=xT[0, k])
            nc.sync.dma_start(out=wv_sb[:, k], in_=wv[k])
        for k in range(NKC):
            nc.gpsimd.dma_start(out=wq_sb[:, k], in_=wq[k])
            nc.sync.dma_start(out=wk_sb[:, k], in_=wk[k])
        for g in range(NG):
            nc.sync.dma_start(out=wo_sb[:, g], in_=wo[g])
        nc.sync.dma_start(out=bqt_sb, in_=bqt[:])
        nc.sync.dma_start(out=bvb_sb, in_=bvb[:])
        nc.sync.dma_start(out=bob_sb, in_=bob[:])
        nc.sync.dma_start(out=msk_sb, in_=msk[:])
        nc.vector.memset(vv[:, :, :, HD:HD + 1], 1.0)
        nc.vector.memset(kT[64:128, :, 0, :], 0.0)
        nc.vector.memset(kT[0:64, :, 1, :], 0.0)
        ones_sb = big.tile([128, HD], MDT, tag="ones")
        nc.vector.memset(ones_sb, 1.0)

        backlog = []
        fillers = []

        def drain(n=1):
            for _ in range(min(n, len(backlog))):
                backlog.pop(0)()

        def pump(n=1):
            while n > 0 and fillers:
                try:
                    next(fillers[0])
                    n -= 1
                except StopIteration:
                    fillers.pop(0)

        def pump_all():
            while fillers:
                try:
                    next(fillers[0])
                except StopIteration:
                    fillers.pop(0)

        def gen_vchain(tci, si, xt):
            st = tci * 4 + si
            psv = pmm.tile([128, 512], F32, tag="mm", name=f"psv{st}")
            for k in range(NKC):
                nc.tensor.matmul(
                    psv, xt[:, k, si * 128:(si + 1) * 128], wv_sb[:, k],
                    start=(k == 0), stop=(k == NKC - 1),
                    skip_group_check=True)
                yield
            nc.vector.tensor_add(
                vv[:, st, :, 0:HD],
                psv.rearrange("p (h d) -> p h d", h=HPC),
                bvb_sb.rearrange("p (h d) -> p h d", h=HPC))

        def gen_qchain(tci, g, xt):
            tsl = slice(tci * 512, (tci + 1) * 512)
            psq = pmm.tile([128, 512], F32, tag="mm", name=f"psq{tci}_{g}")
            for k in range(NKC):
                nc.tensor.matmul(
                    psq, wq_sb[:, k, g * 128:(g + 1) * 128], xt[:, k],
                    start=(k == 0), stop=(k == NKC - 1),
                    skip_group_check=True)
                yield
            nc.vector.tensor_scalar_add(qT[:, g, tsl], psq,
                                        bqt_sb[:, g:g + 1])

        def gen_kchain(tci, g, xt):
            tsl = slice(tci * 512, (tci + 1) * 512)
            psk = pmm.tile([128, 512], F32, tag="mm", name=f"psk{tci}_{g}")
            for k in range(NKC):
                nc.tensor.matmul(
                    psk, wk_sb[:, k, g * 128:(g + 1) * 128], xt[:, k],
                    start=(k == 0), stop=(k == NKC - 1),
                    skip_group_check=True)
                yield
            # k-bias dropped: softmax is invariant to per-q score shifts
            nc.vector.tensor_copy(kT[0:64, g, 0, tsl], psk[0:64])
            nc.vector.tensor_copy(kT[64:128, g, 1, tsl], psk[64:128])

        def gen_ph3(tt):
            tsl = slice(tt * 128, (tt + 1) * 128)
            ot = pot.tile([128, C], F32, tag="ot", name=f"ot{tt}")
            po0 = pmm.tile([128, 512], F32, tag="mm", name=f"po0_{tt}")
            for g in range(NG):
                nc.tensor.matmul(po0, aT[:, g, tsl], wo_sb[:, g, 0:512],
                                 start=(g == 0), stop=(g == NG - 1),
                                 skip_group_check=True)
                yield
            nc.vector.tensor_add(ot[:, 0:512], po0, bob_sb[:, 0:512])
            po1 = pmm.tile([128, 512], F32, tag="mm", name=f"po1_{tt}")
            for g in range(NG):
                nc.tensor.matmul(po1, aT[:, g, tsl], wo_sb[:, g, 512:1024],
                                 start=(g == 0), stop=(g == NG - 1),
                                 skip_group_check=True)
                yield
            nc.vector.tensor_add(ot[:, 512:1024], po1, bob_sb[:, 512:1024])
            nc.sync.dma_start(out=y[tsl], in_=ot)

        def emit_attn(g, qc):
            qcs = qc * 512
            n_st = 4 * (qc + 1)
            pavs = [ppav.tile([HD + 1, 512], F32, tag="pav",
                              name=f"pav{g}_{qc}_{hi}") for hi in range(2)]

            def emit_pav(e, st, off):
                for hi in range(2):
                    nc.tensor.matmul(
                        pavs[hi][:, off:], vv[:, st, 2 * g + hi],
                        e[:, hi, off:],
                        start=(st == 0), stop=(st == n_st - 1),
                        skip_group_check=True)

            def emit_norm(pav, hi):
                lr = pt_.tile([128, 512], MDT, tag="lr",
                              name=f"lr{g}_{qc}_{hi}")
                with nc.allow_low_precision(reason="1/l in bf16"):
                    nc.vector.reciprocal(lr[HD:HD + 1], pav[HD:HD + 1])
                # broadcast 1/l across partitions via PE outer product
                bc = pmm.tile([128, 512], F32, tag="mm",
                              name=f"bc{g}_{qc}_{hi}")
                nc.tensor.matmul(bc[0:HD], ones_sb[HD:HD + 1, :],
                                 lr[HD:HD + 1], start=True, stop=True,
                                 skip_group_check=True)
                bc_sb = pt_.tile([HD, 512], F32, tag="bc_sb",
                                 name=f"bcs{g}_{qc}_{hi}")
                nc.scalar.copy(bc_sb, bc[0:HD])
                if hi == 0:
                    nc.vector.tensor_mul(aT[0:HD, g, qcs:qcs + 512],
                                         pav[0:HD], bc_sb)
                else:
                    tmp = pt_.tile([HD, 512], MDT, tag="tmp",
                                   name=f"tmp{g}_{qc}")
                    nc.vector.tensor_mul(tmp, pav[0:HD], bc_sb)
                    nc.sync.dma_start(out=aT[HD:128, g, qcs:qcs + 512],
                                      in_=tmp)

            for st in range(n_st):
                kk = st - 4 * qc
                off = 128 * kk if kk >= 0 else 0
                sc = psc.tile([128, 2, 512], F32, tag="sc",
                              name=f"sc{g}_{qc}_{st}")
                stsl = slice(st * 128, (st + 1) * 128)
                for hi in range(2):
                    if SCORES_RT:
                        # 64-row tiles: the two heads' matmuls occupy
                        # disjoint PE row groups and can run concurrently
                        po = hi * HD
                        nc.tensor.matmul(
                            sc[:, hi, off:], kT[po:po + HD, g, hi, stsl],
                            qT[po:po + HD, g, qcs + off:qcs + 512],
                            start=True, stop=True, skip_group_check=True)
                    else:
                        nc.tensor.matmul(
                            sc[:, hi, off:], kT[:, g, hi, stsl],
                            qT[:, g, qcs + off:qcs + 512],
                            start=True, stop=True, skip_group_check=True)
                e = pe_.tile([128, 2, 512], MDT, tag="e",
                             name=f"e{g}_{qc}_{st}")
                nc.scalar.activation(out=e[:, :, off:], in_=sc[:, :, off:],
                                     func=EXPF, scale=0.125)
                if kk >= 0:
                    if off:
                        nc.vector.memset(e[:, :, 0:off], 0.0)
                    for hi in range(2):
                        nc.vector.tensor_mul(e[:, hi, off:off + 128],
                                             e[:, hi, off:off + 128], msk_sb)
                backlog.append(
                    lambda e=e, st=st, off=off: emit_pav(e, st, off))
                drain(1)
            backlog.append(lambda pav=pavs[0]: emit_norm(pav, 0))
            backlog.append(lambda pav=pavs[1]: emit_norm(pav, 1))

        # ---------------- fused pipeline ----------------
        import os
        ablate = os.environ.get("KABLATE", "full")
        do_attn = ablate in ("full", "ph12")
        do_ph3 = ablate == "full"
        for tci in range(NTC):
            if tci == 0:
                xt = xt0
            else:
                xt = ph1x.tile([128, NKC, 512], MDT, tag="xt",
                               name=f"xt{tci}")
                for k in range(NKC):
                    nc.gpsimd.dma_start(out=xt[:, k], in_=xT[tci, k])
            emit_ph1(tci, xt)
            if do_ph3 and tci > 0:
                for tt in range(4 * (tci - 1), 4 * tci):
                    emit_ph3(tt)
            if do_attn:
                for g in range(NG):
                    emit_attn(g, tci)
        while backlog:
            drain(1)
        if do_ph3:
            for tt in range(4 * (NTC - 1), 4 * NTC):
                emit_ph3(tt)
        elif ablate == "ph1":
            # flush something touching qT/kT/vv so nothing is dead-code'd
            ot = pot.tile([128, C], F32, tag="ot", name="otp1")
            nc.vector.tensor_copy(ot[:, 0:512], qT[:, 0, 0:512])
            nc.sync.dma_start(out=y[0:128], in_=ot)
        elif ablate == "ph12":
            ot = pot.tile([128, C], F32, tag="ot", name="otp2")
            nc.vector.tensor_copy(ot[:, 0:512], aT[:, 0, 0:512])
            nc.sync.dma_start(out=y[0:128], in_=ot)


def _split_matmul_waits(nc):
    """walrus codegen allows only ONE sync-wait per engine instruction.
    Move surplus waits of any multi-wait instruction onto preceding
    same-engine NoOps (one wait each) — engine dispatch is in-order, so
    the NoOps gate the instruction."""
    from concourse import mybir

    inst_noop_cls = None
    for fn in nc.m.functions:
        for blk in fn.blocks:
            new_insts = []
            for inst in blk.instructions:
                si = getattr(inst, "sync_info", None)
                if (si is not None
                        and si.on_wait and len(si.on_wait) > 1):
                    if inst_noop_cls is None:
                        import bass_rust
                        inst_noop_cls = bass_rust.InstNoOp
                    waits = list(si.on_wait)
                    si.on_wait = waits[-1:]
                    for w in waits[:-1]:  # one wait per NoOp (HW limit)
                        nop = inst_noop_cls(
                            name=nc.get_next_instruction_name(), ins=[], outs=[])
                        nop.engine = inst.engine
                        nop.sync_info = mybir.SyncInfo(on_wait=[w], on_update=[])
                        nc.register_instruction(nop)
                        new_insts.append(nop)
                new_insts.append(inst)
            blk.instructions[:] = new_insts


def to_bf16(a):
    import ml_dtypes
    return np.ascontiguousarray(
        np.ascontiguousarray(a, np.float32).astype(ml_dtypes.bfloat16))


def prepare_inputs(inputs):
    """Per-core input maps (host-side sharding + layout munging)."""
    x = np.asarray(inputs["x"], np.float32)
    Wq = np.asarray(inputs["Wq"], np.float32)
    bq = np.asarray(inputs["bq"], np.float32)
    Wk = np.asarray(inputs["Wk"], np.float32)
    Wv = np.asarray(inputs["Wv"], np.float32)
    bv = np.asarray(inputs["bv"], np.float32)
    Wo = np.asarray(inputs["Wo"], np.float32)
    bo = np.asarray(inputs["bo"], np.float32)

    p = np.arange(128)[:, None]
    f = np.arange(128)[None, :]
    msk = to_bf16((p <= f).astype(np.float32))  # [128, 128] triangular

    in_maps = []
    for c in range(NCORES):
        b, hg = c // 2, c % 2
        rows = slice(hg * DC, (hg + 1) * DC)
        in_maps.append({
            "xT": np.ascontiguousarray(
                to_bf16(x[b].T).reshape(NKC, 128, NTC, 512)
                .transpose(2, 0, 1, 3)),
            "wq": to_bf16(Wq[rows, :].T).reshape(NKC, 128, DC),
            "wk": to_bf16(Wk[rows, :].T).reshape(NKC, 128, DC),
            "wv": to_bf16(Wv[rows, :].T).reshape(NKC, 128, DC),
            "wo": to_bf16(Wo[:, rows].T).reshape(NG, 128, C),
            "bqt": np.ascontiguousarray(bq[rows].reshape(NG, 128).T),
            "bvb": np.tile(bv[rows][None, :], (128, 1)),
            "bob": (np.tile(bo[None, :], (128, 1)) if hg == 0
                    else np.zeros((128, C), np.float32)),
            "msk": msk,
        })
    return in_maps


def gather_outputs(results):
    ys = [np.asarray(r["y"], np.float32) for r in results]
    return np.stack([ys[2 * b] + ys[2 * b + 1] for b in range(B)], axis=0)


def kernel(**inputs):
    nc = build()
    in_maps = prepare_inputs(inputs)
    res = run_bass_kernel_spmd(nc, in_maps, core_ids=list(range(NCORES)))
    return gather_outputs(res.results)
